# revision 21
# baseline (speedup 1.0000x reference)
"""QMIX-style MixingNetwork Trainium2 kernel (fp8 DoubleRow + custom DVE).

Shapes (hardcoded from the problem spec):
  B, S, A, C, E, H = 256, 512, 8, 256, 64, 256
  agent_q_values [B, S, A], central_states [B, S, C] -> out [B, S, 1]

Strategy: pure data parallel over samples across 8 NeuronCores
(16384 samples/core), transposed activation layout (features on
partitions, samples on the free dim), 16 chunks of 1024 samples.

Per chunk:
  GEMM1 (fp8e4m3 DoubleRow, K=256 in one MM): z[832, n] = wcat.T @ x
  ELU:   hp = elu(z+b)+1 = max(z+b+1, min(exp(z+b), 1))
         exp on ACT, combine via custom DVE op ELU_P1 (one PSUM read)
  GEMM2 (fp8 DR): w1p[512, n] from hp; |.|+bias on ACT (Abs), q-mult on
         Pool (gpsimd), a-sum via fp8 DR matmul against selection matrix
  tail:  w2p = w2b.T@hp (DR), dotp = relu(psh+bb1)*|w2p+b2b'| in ONE
         custom DVE op, joint = wb2b.T@hp45 + ones.T@dotp (PSUM
         accumulate), out = joint + const (ACT/DVE alternating).

The "+1" offset trick: hp = elu+1; the constant offset is folded into
the next layer's bias host-side using the QUANTIZED weights
(b' = b - colsum(fp8(W))).
"""

import os
import sys

for _p in ("/opt/trn_rl_repo", "/root/.axon_site/_ro/trn_rl_repo"):
    if os.path.isdir(_p) and _p not in sys.path:
        sys.path.append(_p)

from contextlib import ExitStack

import numpy as np

import concourse.bass as bass
import concourse.mybir as mybir
import concourse.tile as tile
from concourse import bacc
from concourse.bass_utils import run_bass_kernel_spmd

import concourse.dve_ops as dve_ops_mod
from concourse.dve_ops import DveOp
from concourse.dve_spec import (
    Spec, Src0, Src1, C0, C1, C2, maxx, minn, relu, Zero, One, lower, _has_src1,
)
from concourse.dve_uop import DveOpSpec

B, S, A, C, E, H = 256, 512, 8, 256, 64, 256
N_CORES = 8
NC_SAMPLES = B * S // N_CORES        # 16384 samples per core
CHUNK = 1024                         # samples per chunk
HALF = 512                           # psum-bank half-chunk
N_CHUNKS = NC_SAMPLES // CHUNK       # 16
F = 3 * H + E                        # 832 fused GEMM1 output dim
AE = A * E                           # 512

FP32 = mybir.dt.float32
BF16 = mybir.dt.bfloat16
FP8 = mybir.dt.float8e4
ALU = mybir.AluOpType
AF = mybir.ActivationFunctionType
DR = mybir.MatmulPerfMode.DoubleRow

# bias_sb column layout
COL_B = 0       # 6 cols: b       (exp bias), f=0..5
COL_BP1 = 6     # 6 cols: b + 1   (ELU_P1 linear-branch bias)
COL_B1B = 12    # 4 cols: b1b - colsum(fp8(w1b)), m=0..3
COL_BB1 = 16    # 1 col:  bb1 in rows 0:64
COL_B2B = 17    # 1 col:  b2b - colsum(fp8(w2b)) in rows 0:64
N_BIAS_COLS = 18


def _register_op(name, spec):
    if name in dve_ops_mod._SUB_OPCODE_FOR_NAME:
        return next(op for op in dve_ops_mod.OPS if op.name == name)
    row = dve_ops_mod._CUSTOM_DVE_ROW_BASE + len(dve_ops_mod.OPS)
    assert row < 0x20
    dve_ops_mod._SUB_OPCODE_FOR_NAME[name] = row
    shas = {}
    for ver in ("v3", "v4"):
        uops = lower(spec, ver=ver)
        s = DveOpSpec(name=name, opcode=row, uops=uops, rd1_en=_has_src1(spec))
        shas[ver] = s.sha(ver)
    op = DveOp(name, spec, subdim=False, uops_sha=shas)
    dve_ops_mod.OPS.append(op)
    dve_ops_mod.CUSTOM_DVE_SPECS[name] = spec
    return op


# el = elu(z + c0) = max(z + c0, min(e, c2) - 1); c0 = b, c2 = 1.0
ELU_SH = _register_op(
    "ELU_SH_ANT", Spec(body=maxx(Src0 + C0, minn(Src1, C2) - One)))
# pr = |z + c0| * q  (abs via max(t, -t))
_t = Src0 + C0
ABS_MUL = _register_op("ABS_MUL_ANT", Spec(body=maxx(_t, Zero - _t) * Src1))
# dotp = relu(psh + c0) * w2t   (only one PSUM operand allowed on DVE)
RELU_MUL = _register_op("RELU_MUL_ANT", Spec(body=relu(Src0 + C0) * Src1))


def _build_nc():
    nc = bacc.Bacc("TRN2", target_bir_lowering=False, debug=False)

    xdr = nc.dram_tensor("xdr", [128, 2 * NC_SAMPLES], BF16, kind="ExternalInput")
    qdr = nc.dram_tensor("qdr", [128, 4 * NC_SAMPLES], BF16, kind="ExternalInput")
    wcat = nc.dram_tensor("wcat", [128, 2 * F], BF16, kind="ExternalInput")
    w1b = nc.dram_tensor("w1b", [128, 2 * AE], BF16, kind="ExternalInput")
    w2b = nc.dram_tensor("w2b", [128, 2 * E], BF16, kind="ExternalInput")
    wb2b = nc.dram_tensor("wb2b", [128, 2 * 32], BF16, kind="ExternalInput")
    selp = nc.dram_tensor("selp", [128, 2 * E], FP8, kind="ExternalInput")
    ones = nc.dram_tensor("ones", [E, 1], BF16, kind="ExternalInput")
    biases = nc.dram_tensor("biases", [128, N_BIAS_COLS], FP32,
                            kind="ExternalInput")
    ob = nc.dram_tensor("ob", [2, 1], FP32, kind="ExternalInput")
    out = nc.dram_tensor("out", [1, NC_SAMPLES], FP32, kind="ExternalOutput")

    with ExitStack() as ctx:
        tc = ctx.enter_context(tile.TileContext(nc))
        singles = ctx.enter_context(tc.tile_pool(name="singles", bufs=1))
        xpool = ctx.enter_context(tc.tile_pool(name="xpool", bufs=2))
        qpool = ctx.enter_context(tc.tile_pool(name="qpool", bufs=2))
        epool = ctx.enter_context(tc.tile_pool(name="epool", bufs=3))
        hpool = ctx.enter_context(tc.tile_pool(name="hpool", bufs=2))
        apool = ctx.enter_context(tc.tile_pool(name="apool", bufs=2))
        rpool = ctx.enter_context(tc.tile_pool(name="rpool", bufs=2))
        dpool = ctx.enter_context(tc.tile_pool(name="dpool", bufs=2))
        zps = ctx.enter_context(tc.tile_pool(name="zps", bufs=2, space="PSUM"))
        pwps = ctx.enter_context(tc.tile_pool(name="pwps", bufs=1, space="PSUM"))
        shps = ctx.enter_context(tc.tile_pool(name="shps", bufs=1, space="PSUM"))

        # ---- preload weights/constants ----
        wcat_sb = []
        for j in range(2):
            t = singles.tile([128, F], BF16, tag=f"wcat{j}")
            nc.sync.dma_start(out=t, in_=wcat[:, j * F:(j + 1) * F])
            wcat_sb.append(t)
        w1b_sb = []
        for j in range(2):
            t = singles.tile([128, AE], BF16, tag=f"w1b{j}")
            nc.sync.dma_start(out=t, in_=w1b[:, j * AE:(j + 1) * AE])
            w1b_sb.append(t)
        w2b_sb = []
        for j in range(2):
            t = singles.tile([128, E], BF16, tag=f"w2b{j}")
            nc.sync.dma_start(out=t, in_=w2b[:, j * E:(j + 1) * E])
            w2b_sb.append(t)
        wb2b_sb = []
        for j in range(2):
            t = singles.tile([128, 32], BF16, tag=f"wb2b{j}")
            nc.sync.dma_start(out=t, in_=wb2b[:, j * 32:(j + 1) * 32])
            wb2b_sb.append(t)
        sel_sb = singles.tile([128, 2, E], FP8, tag="selp")
        nc.sync.dma_start(out=sel_sb, in_=selp[:, :])
        ones_sb = singles.tile([E, 1], BF16, tag="ones")
        nc.sync.dma_start(out=ones_sb, in_=ones[:, :])
        bias_sb = singles.tile([128, N_BIAS_COLS], FP32, tag="bias")
        nc.sync.dma_start(out=bias_sb, in_=biases[:, :])
        ob_sb = singles.tile([2, 1], FP32, tag="ob")
        nc.sync.dma_start(out=ob_sb, in_=ob[:, :])
        out_sb = singles.tile([1, NC_SAMPLES], FP32, tag="out")

        def bcol(col, parts=128):
            return bias_sb[0:parts, col:col + 1]

        for ci in range(N_CHUNKS):
            cs = slice(ci * CHUNK, (ci + 1) * CHUNK)

            xts = []
            for j in range(2):
                t = xpool.tile([128, CHUNK], BF16, tag=f"xt{j}")
                nc.sync.dma_start(
                    out=t, in_=xdr[:, (2 * ci + j) * CHUNK:(2 * ci + j + 1) * CHUNK])
                xts.append(t)
            qb = qpool.tile([128, 4, CHUNK], BF16, tag="qb")
            nc.gpsimd.dma_start(out=qb, in_=qdr[:, ci * 4 * CHUNK:(ci + 1) * 4 * CHUNK])

            hp01 = hpool.tile([128, 2, CHUNK], BF16, tag="hp01")
            hp23 = hpool.tile([128, 2, CHUNK], BF16, tag="hp23")
            hp45 = hpool.tile([128, 2, CHUNK], BF16, tag="hp45")
            hps = [hp01, hp23, hp45]

            # ---- GEMM1 + ELU ----
            for f in range(6):
                z = zps.tile([128, CHUNK], FP32, tag="z")
                for h in range(2):
                    for j in range(2):
                        nc.tensor.matmul(
                            z[:, h * HALF:(h + 1) * HALF],
                            wcat_sb[j][:, f * 128:(f + 1) * 128],
                            xts[j][:, h * HALF:(h + 1) * HALF],
                            start=(j == 0), stop=(j == 1))
                e = epool.tile([128, CHUNK], BF16, tag="e")
                nc.scalar.activation(e, z, AF.Exp, bias=bcol(COL_B + f))
                nc.vector._custom_dve(
                    ELU_SH, out=hps[f // 2][:, f % 2, :], in0=z, in1=e,
                    s0=bcol(COL_B + f), s1=0.0, imm2=1.0)

            # b1 branch -> psh (start of hidden accumulation)
            shp = shps.tile([64, CHUNK], FP32, tag="psh")
            psh = shp[0:64, :]
            for h in range(2):
                for j in range(2):
                    nc.tensor.matmul(
                        psh[:, h * HALF:(h + 1) * HALF],
                        wcat_sb[j][:, 768:832],
                        xts[j][:, h * HALF:(h + 1) * HALF],
                        start=(j == 0), stop=False,
                        skip_group_check=True)



            # ---- GEMM2 + abs + q-mult ----
            pr = rpool.tile([128, 4, CHUNK], FP8, tag="pr")
            for m in range(4):
                pw = pwps.tile([128, CHUNK], FP32, tag="pw")
                for h in range(2):
                    for j in range(2):
                        nc.tensor.matmul(
                            pw[:, h * HALF:(h + 1) * HALF],
                            w1b_sb[j][:, m * 128:(m + 1) * 128],
                            hp01[:, j, h * HALF:(h + 1) * HALF],
                            start=(j == 0), stop=(j == 1))
                if m >= 2 + (ci % 2):
                    # fused |pw + b| * q on DVE
                    nc.vector._custom_dve(
                        ABS_MUL, out=pr[:, m, :], in0=pw, in1=qb[:, m, :],
                        s0=bcol(COL_B1B + m), s1=0.0, imm2=0.0)
                else:
                    aw = apool.tile([128, CHUNK], BF16, tag="aw")
                    nc.scalar.activation(aw, pw, AF.Abs, bias=bcol(COL_B1B + m))
                    nc.gpsimd.tensor_tensor(pr[:, m, :], aw, qb[:, m, :],
                                            ALU.mult)

            # a-sum: 2 DR matmuls per half over m-pairs
            for h in range(2):
                for mp in range(2):
                    nc.tensor.matmul(
                        psh[:, h * HALF:(h + 1) * HALF],
                        sel_sb[:, :, :],
                        pr[:, 2 * mp:2 * mp + 2, h * HALF:(h + 1) * HALF],
                        start=False, stop=(mp == 1), perf_mode=DR,
                        skip_group_check=True)

            # w2 branch
            w2p = pwps.tile([64, CHUNK], FP32, tag="pw")
            for h in range(2):
                for j in range(2):
                    nc.tensor.matmul(
                        w2p[:, h * HALF:(h + 1) * HALF],
                        w2b_sb[j],
                        hp23[:, j, h * HALF:(h + 1) * HALF],
                        start=(j == 0), stop=(j == 1))

            # w2t = |w2p + b2b'| on ACT; dotp = relu(psh + bb1) * w2t on DVE
            w2t = dpool.tile([64, CHUNK], BF16, tag="w2t")
            nc.scalar.activation(w2t, w2p, AF.Abs, bias=bcol(COL_B2B, 64))
            dotp = dpool.tile([64, CHUNK], BF16, tag="dotp")
            nc.vector._custom_dve(
                RELU_MUL, out=dotp, in0=psh, in1=w2t,
                s0=bcol(COL_BB1, 64), s1=0.0, imm2=0.0)

            b2p = shps.tile([32, CHUNK], FP32, tag="psh")
            for h in range(2):
                for j in range(2):
                    nc.tensor.matmul(
                        b2p[:, h * HALF:(h + 1) * HALF],
                        wb2b_sb[j],
                        hp45[:, j, h * HALF:(h + 1) * HALF],
                        start=(j == 0), stop=False,
                        skip_group_check=True)
            for h in range(2):
                nc.tensor.matmul(
                    b2p[0:1, h * HALF:(h + 1) * HALF],
                    ones_sb,
                    dotp[:, h * HALF:(h + 1) * HALF],
                    start=False, stop=True,
                    skip_group_check=True)

            # out = joint + bb2b
            nc.scalar.activation(
                out_sb[0:1, cs],
                b2p[0:1, :], AF.Identity,
                bias=ob_sb[0:1, 0:1])

        nc.sync.dma_start(out=out[:, :], in_=out_sb)

    nc.compile()
    return nc


_NC_CACHE = None


def _get_nc():
    global _NC_CACHE
    if _NC_CACHE is None:
        _NC_CACHE = _build_nc()
    return _NC_CACHE


def _dr2(a):
    """[256, X] -> DoubleRow layout [128, 2*X] ([p, j*X + x] = a[j*128+p, x])."""
    x = a.shape[1]
    return np.ascontiguousarray(
        a.reshape(2, 128, x).transpose(1, 0, 2).reshape(128, 2 * x))


def _prep_inputs(agent_q_values, central_states, weights):
    import ml_dtypes
    f8 = np.dtype(ml_dtypes.float8_e4m3)
    bf = np.dtype(ml_dtypes.bfloat16)

    st = central_states.reshape(B * S, C)
    q = agent_q_values.reshape(B * S, A)

    (w1a, b1a, w1b, b1b, w2a, b2a, w2b, b2b,
     wb1, bb1, wb2a, bb2a, wb2b, bb2b) = weights

    wcat = np.concatenate([w1a, w2a, wb2a, wb1], axis=1)          # [C, 832]
    bcat = np.concatenate([b1a, b2a, bb2a])                        # [768]

    wb2bw = np.zeros((C, 32), np.float32)
    wb2bw[:, 0] = wb2b[:, 0]

    bias_pack = np.zeros((128, N_BIAS_COLS), np.float32)
    for f in range(6):
        seg = bcat[f * 128:(f + 1) * 128]
        bias_pack[:, COL_B + f] = seg
    # hp stores the true elu hidden: plain original biases
    for m in range(4):
        bias_pack[:, COL_B1B + m] = b1b[m * 128:(m + 1) * 128]
    bias_pack[0:64, COL_BB1] = bb1
    bias_pack[0:64, COL_B2B] = b2b
    ob = np.array([[bb2b[0]], [bb2b[0]]], np.float32)

    selp = np.zeros((128, 2, E), np.float32)
    p = np.arange(128)
    for j in range(2):
        selp[p, j, p % 64] = 1.0
    ones = np.ones((E, 1), np.float32)

    shared = dict(
        wcat=_dr2(wcat).astype(bf),
        w1b=_dr2(w1b).astype(bf),
        w2b=_dr2(w2b).astype(bf),
        wb2b=_dr2(wb2bw).astype(bf),
        selp=np.ascontiguousarray(selp.reshape(128, 2 * E)).astype(f8),
        ones=ones.astype(bf),
        biases=bias_pack, ob=ob,
    )

    in_maps = []
    for c in range(N_CORES):
        sl = slice(c * NC_SAMPLES, (c + 1) * NC_SAMPLES)
        xt_c = st[sl].T                                         # [C, n]
        xdr = xt_c.reshape(2, 128, N_CHUNKS, CHUNK).transpose(
            1, 2, 0, 3).reshape(128, 2 * NC_SAMPLES)
        q_c = q[sl].T                                           # [A, n]
        qfull = np.repeat(q_c, E, axis=0)                       # [512, n]
        qdr = qfull.reshape(4, 128, N_CHUNKS, CHUNK).transpose(
            1, 2, 0, 3).reshape(128, 4 * NC_SAMPLES)
        in_maps.append(dict(
            xdr=np.ascontiguousarray(xdr).astype(bf),
            qdr=np.ascontiguousarray(qdr).astype(bf),
            **shared))
    return in_maps


def kernel(agent_q_values, central_states,
           w1a, b1a, w1b, b1b, w2a, b2a, w2b, b2b,
           wb1, bb1, wb2a, bb2a, wb2b, bb2b, _trace=False, _result_box=None):
    nc = _get_nc()
    weights = (w1a, b1a, w1b, b1b, w2a, b2a, w2b, b2b,
               wb1, bb1, wb2a, bb2a, wb2b, bb2b)
    weights = tuple(np.asarray(w, np.float32) for w in weights)
    in_maps = _prep_inputs(
        np.asarray(agent_q_values, np.float32),
        np.asarray(central_states, np.float32), weights)

    res = run_bass_kernel_spmd(nc, in_maps, core_ids=list(range(N_CORES)),
                               trace=_trace)
    if _result_box is not None:
        _result_box.append(res)

    out = np.concatenate(
        [res.results[c]["out"].reshape(NC_SAMPLES) for c in range(N_CORES)])
    return out.reshape(B, S, 1).astype(np.float32)
